# revision 31
# baseline (speedup 1.0000x reference)
"""AdmittanceGNN (3-layer edge-attention GNN) on 8 Trainium2 NeuronCores.

Strategy (dst-sharded, v2):
  - Nodes sharded into 8 contiguous ranges (6272/core, padded to 50176).
    Each core owns all edges whose dst falls in its range, so segment sums
    are core-local.
  - Per layer, each core computes per-node tables for ITS nodes:
      u = (x@Wn)@W1a   (dst-side attention term, stays in SBUF)
      v = (x@Wn)@W1b, w = (x@Wn)@We1  (src-side, [v|w] rows AllGathered)
  - Per-edge v/w fetched with batched dma_gather (512B rows; int16 indices
    windowed by table halves).  The dst-side u is NOT gathered from DRAM:
    since each 128-slot tile's dsts live in one 128-node block, u[dst] is
    produced on-chip as S1T^T @ u_block where S1T[n, slot] = (n == off(slot))
    is built per-window from a partition_broadcast of the offs stripe plus
    one is_equal.
  - Attention: a1 = relu(u[dst]+v[src]); att = sigmoid(sum(a1*w2)).
  - Scatter: satt[slot, n] = (n == off(slot))*att(slot); per tile
    acc[block] += satt^T @ [w|ea]_gathered on the PE (PSUM accumulation;
    edge_attr is inserted into the gathered rows' pad so the P2 factor
    rides the same matmul as 2 extra columns).  eat = P2 @ We2 is applied
    per block on the DVE using host-partition-broadcast We2 rows.
  - LayerNorm stats via bn_stats/bn_aggr per block; sqrt/reciprocal and the
    (x-mu)*rstd*+relu+residual application deferred to layer end (keeps the
    act engine on the Sigmoid table set during the edge pass).
"""
import numpy as np
import ml_dtypes

import concourse.bass as bass
import concourse.bacc as bacc
import concourse.tile as tile
import concourse.mybir as mybir
from concourse import bass_utils

P = 128
D = 128
H = 64
LN_EPS = 1e-5

f32 = mybir.dt.float32
bf16 = mybir.dt.bfloat16
i16 = mybir.dt.int16
BF = ml_dtypes.bfloat16

import os
VARIANT = os.environ.get("KVARIANT", "full")
# full | nocoll | tablesonly | tabag | gonly | nou

AL = mybir.AluOpType
AF = mybir.ActivationFunctionType


class Cfg:
    def __init__(self, N, E, L, ncores, bpc, window_b=2, half=32768):
        self.N, self.E, self.L, self.ncores = N, E, L, ncores
        self.bpc = bpc                      # blocks of 128 nodes per core
        self.nodes_pc = bpc * P
        self.npad = ncores * self.nodes_pc
        self.window_b = window_b
        self.half = half                    # src-index window split
        assert self.npad >= N
        self.windows = [list(range(i, min(i + window_b, bpc)))
                        for i in range(0, bpc, window_b)]


REAL = Cfg(N=50000, E=640000, L=3, ncores=8, bpc=49, window_b=2, half=32768)


# ---------------------------------------------------------------- host prep
def prep_edges(cfg, edge_index):
    """Bucket/sort/pad edges; build per-core slot arrays + shared layout.

    Slot order (identical across cores): for each window w (window_b blocks):
      [lo-seg(b0) | lo-seg(b1) | ... | hi-seg(b0) | hi-seg(b1) | ...]
    each segment padded to a multiple of 128. Slot s -> tile t=s//128,
    partition p=s%128.
    """
    src = np.asarray(edge_index[0], dtype=np.int64)
    dst = np.asarray(edge_index[1], dtype=np.int64)
    E = len(src)
    nc_, bpc, npc = cfg.ncores, cfg.bpc, cfg.nodes_pc

    core = dst // npc
    loc = dst - core * npc
    blk = loc // P
    off = loc % P
    is_lo = src < cfg.half

    cnt = np.zeros((nc_, bpc, 2), dtype=np.int64)
    np.add.at(cnt, (core, blk, 1 - is_lo.astype(np.int64)), 1)
    tiles = np.maximum(0, -(-cnt.max(axis=0) // P))        # [bpc, 2] shared
    seg_start = np.zeros((bpc, 2), dtype=np.int64)
    win_meta = []
    pos = 0
    gt = 0
    for wblocks in cfg.windows:
        w = dict(blocks=wblocks, slot0=pos, tile0=gt)
        lo_tiles = []
        hi_tiles = []
        for half_i in (0, 1):
            for b in wblocks:
                seg_start[b, half_i] = pos
                t = int(tiles[b, half_i])
                (lo_tiles if half_i == 0 else hi_tiles).append((b, t))
                pos += t * P
                gt += t
        w["s_lo"] = sum(t for _, t in lo_tiles) * P
        w["s_hi"] = sum(t for _, t in hi_tiles) * P
        w["tiles"] = []
        for b, t in lo_tiles + hi_tiles:
            for _ in range(t):
                w["tiles"].append(b)
        win_meta.append(w)
    tot_slots = pos
    tot_tiles = gt

    blk_tiles = [[] for _ in range(bpc)]
    for w in win_meta:
        for ti, b in enumerate(w["tiles"]):
            blk_tiles[b].append(w["tile0"] + ti)

    out = []
    for c in range(nc_):
        m = core == c
        srcc, blkc, offc, loi = src[m], blk[m], off[m], is_lo[m]
        eidc = np.nonzero(m)[0]
        srcidx = np.zeros(tot_slots, dtype=np.int64)
        offs = np.full(tot_slots, -1.0, dtype=np.float32)
        eslot = np.full(tot_slots, -1, dtype=np.int64)   # edge id per slot
        h = 1 - loi.astype(np.int64)
        order = np.lexsort((srcc, h, blkc))
        gkey = (blkc * 2 + h)[order]
        first = np.r_[True, gkey[1:] != gkey[:-1]]
        idxs = np.arange(len(gkey))
        grp_start = idxs[first]
        rank = idxs - np.repeat(grp_start, np.diff(np.r_[grp_start, len(gkey)]))
        s = seg_start[blkc[order], h[order]] + rank
        srcidx[s] = srcc[order] - np.where(h[order] == 1, cfg.half, 0)
        offs[s] = offc[order].astype(np.float32)
        eslot[s] = eidc[order]
        out.append(dict(srcidx=srcidx, offs=offs, eslot=eslot))

    meta = dict(win=win_meta, tot_slots=tot_slots, tot_tiles=tot_tiles,
                blk_tiles=blk_tiles)
    return out, meta


def wrap16(vals):
    """Wrap a 1-D int index array into the [128, S/16] int16 layout
    (logical position j lives at [j % 16, j // 16], replicated to 128
    partitions for the descriptor-generating Q7 cores)."""
    n = len(vals)
    S = -(-n // 16)
    flat = np.zeros(16 * S, dtype=np.int16)
    flat[:n] = vals.astype(np.int16)
    arr = np.ascontiguousarray(flat.reshape(S, 16).T)
    return np.tile(arr, (8, 1))


def build_core_inputs(cfg, meta, percore, x_pad):
    ins = []
    for c in range(cfg.ncores):
        pc = percore[c]
        vw_cols = []
        for w in meta["win"]:
            s0, sl, sh = w["slot0"], w["s_lo"], w["s_hi"]
            sidx = pc["srcidx"][s0:s0 + sl + sh]
            vw_cols.append(wrap16(sidx[:sl]))
            if sh:
                vw_cols.append(wrap16(sidx[sl:]))
        vwidx = np.concatenate(vw_cols, axis=1) if vw_cols else \
            np.zeros((128, 1), np.int16)
        tt = meta["tot_tiles"]
        offs = pc["offs"].reshape(tt, P).T.copy()          # [128, tt]
        offs_flat = pc["offs"].reshape(1, -1).astype(BF)   # [1, tot_slots]
        x_own = x_pad[c * cfg.nodes_pc:(c + 1) * cfg.nodes_pc]
        xrows = x_own.reshape(cfg.bpc, P, D).transpose(1, 0, 2) \
            .reshape(P, cfg.bpc * D)
        ins.append(dict(vwidx=vwidx, offs=offs, offs_flat=offs_flat,
                        xrows=np.ascontiguousarray(xrows, dtype=np.float32)))
    return ins


# ---------------------------------------------------------------- device code
def build_nc(cfg, meta):
    nc = bacc.Bacc("TRN2", target_bir_lowering=False, debug=False,
                   num_devices=cfg.ncores, num_swdge_queues=4)
    L, bpc, npc = cfg.L, cfg.bpc, cfg.nodes_pc
    tt = meta["tot_tiles"]
    ts = meta["tot_slots"]
    vw_icols = sum(w["s_lo"] // 16 + w["s_hi"] // 16 for w in meta["win"])
    max_sall = max(w["s_lo"] + w["s_hi"] for w in meta["win"])
    max_tw = max_sall // P

    # ---------------- I/O
    xrows_d = nc.dram_tensor("xrows", [P, bpc * D], f32, kind="ExternalInput")
    vwidx_d = nc.dram_tensor("vwidx", [P, vw_icols], i16, kind="ExternalInput")
    offs_d = nc.dram_tensor("offs", [P, tt], f32, kind="ExternalInput")
    offsf_d = nc.dram_tensor("offs_flat", [1, ts], bf16, kind="ExternalInput")
    ea_d = nc.dram_tensor("ea2", [P, 2 * tt], bf16, kind="ExternalInput")
    wn_d = nc.dram_tensor("wn", [L, D, D], bf16, kind="ExternalInput")
    w1a_d = nc.dram_tensor("w1a", [L, D, H], bf16, kind="ExternalInput")
    w1b_d = nc.dram_tensor("w1b", [L, D, H], bf16, kind="ExternalInput")
    we1_d = nc.dram_tensor("we1", [L, D, D], bf16, kind="ExternalInput")
    we2b_d = nc.dram_tensor("we2b", [L, P, 2 * D], bf16, kind="ExternalInput")
    w2r_d = nc.dram_tensor("w2r", [L, P, H], bf16, kind="ExternalInput")
    ident_d = nc.dram_tensor("ident", [P, P], f32, kind="ExternalInput")
    iota_d = nc.dram_tensor("iota", [P, P], bf16, kind="ExternalInput")
    iotac_d = nc.dram_tensor("iotac", [P, 1], f32, kind="ExternalInput")
    y_d = nc.dram_tensor("y", [P, bpc * D], f32, kind="ExternalOutput")

    # DRAM scratch (double-buffered across layers)
    vw_own = [nc.dram_tensor(f"vw_own{i}", [npc, 256], bf16, kind="Internal")
              for i in range(2)]
    aspace = "Shared" if cfg.ncores > 4 else "Local"
    vw_full = [nc.dram_tensor(f"vw_full{i}", [cfg.npad, 256], bf16,
                              kind="Internal", addr_space=aspace)
               for i in range(2)]

    with tile.TileContext(nc) as tc:
        with (
            tc.tile_pool(name="res", bufs=1) as res,
            tc.tile_pool(name="wp", bufs=2) as wp,
            tc.tile_pool(name="win", bufs=3) as wnp,
            tc.tile_pool(name="satt", bufs=6) as sap,
            tc.tile_pool(name="small", bufs=4) as smp,
            tc.tile_pool(name="psA", bufs=4, space="PSUM") as psA,
            tc.tile_pool(name="psB", bufs=2, space="PSUM") as psB,
            tc.tile_pool(name="psD", bufs=2, space="PSUM") as psD,
        ):
            # ---------------- resident tiles
            xrows = res.tile([P, bpc * D], f32)
            nc.sync.dma_start(xrows[:], xrows_d[:])
            vwidx = res.tile([P, vw_icols], i16)
            nc.sync.dma_start(vwidx[:], vwidx_d[:])
            offs = res.tile([P, tt], f32)
            nc.sync.dma_start(offs[:], offs_d[:])
            ea = res.tile([P, 2 * tt], bf16)
            nc.sync.dma_start(ea[:], ea_d[:])
            ident = res.tile([P, P], f32)
            nc.sync.dma_start(ident[:], ident_d[:])
            iota = res.tile([P, P], bf16)
            nc.sync.dma_start(iota[:], iota_d[:])
            iotac = res.tile([P, 1], f32)
            nc.sync.dma_start(iotac[:], iotac_d[:])
            eps_sb = res.tile([P, 1], f32)
            nc.vector.memset(eps_sb[:], LN_EPS)
            u_sb = res.tile([P, bpc * H], bf16)     # u table, per-layer
            osl_all = res.tile([P, bpc * D], f32)   # pre-LN outputs
            mv = res.tile([P, 2 * bpc], f32)        # bn (mean, var) pairs

            for l in range(L):
                pb = l % 2
                # ---- layer weights
                wn_sb = wp.tile([D, D], bf16, tag="wn")
                nc.sync.dma_start(wn_sb[:], wn_d[l])
                w1a_sb = wp.tile([D, H], bf16, tag="w1a")
                nc.sync.dma_start(w1a_sb[:], w1a_d[l])
                w1b_sb = wp.tile([D, H], bf16, tag="w1b")
                nc.sync.dma_start(w1b_sb[:], w1b_d[l])
                we1_sb = wp.tile([D, D], bf16, tag="we1")
                nc.sync.dma_start(we1_sb[:], we1_d[l])
                we2b_sb = wp.tile([P, 2 * D], bf16, tag="we2b")
                nc.sync.dma_start(we2b_sb[:], we2b_d[l])
                w2r_sb = wp.tile([P, H], bf16, tag="w2r")
                nc.sync.dma_start(w2r_sb[:], w2r_d[l])

                # ---- tables (batched 4 blocks per PSUM bank)
                for g0 in range(0, bpc, 4):
                    gl = min(4, bpc - g0)
                    ncols = gl * P
                    tp = psD.tile([P, 512], f32, space="PSUM", tag="tp")
                    for j in range(gl):
                        nc.tensor.transpose(
                            tp[:, j * P:(j + 1) * P],
                            xrows[:, (g0 + j) * D:(g0 + j + 1) * D], ident[:])
                    xt = smp.tile([P, 512], bf16, tag="xt", bufs=2)
                    nc.vector.tensor_copy(xt[:, 0:ncols], tp[:, 0:ncols])
                    hps = psB.tile([P, 512], f32, space="PSUM", tag="ug")
                    nc.tensor.matmul(hps[:, 0:ncols], lhsT=wn_sb[:],
                                     rhs=xt[:, 0:ncols], start=True, stop=True)
                    ht = smp.tile([P, 512], bf16, tag="ht", bufs=2)
                    nc.vector.tensor_copy(ht[:, 0:ncols], hps[:, 0:ncols])
                    for j in range(gl):
                        b = g0 + j
                        hsl = ht[:, j * P:(j + 1) * P]
                        puvw = psB.tile([P, 512], f32, space="PSUM", tag="ug")
                        nc.tensor.matmul(puvw[:, 0:H], lhsT=hsl, rhs=w1a_sb[:],
                                         start=True, stop=True)
                        nc.tensor.matmul(puvw[:, H:2 * H], lhsT=hsl,
                                         rhs=w1b_sb[:], start=True, stop=True)
                        nc.tensor.matmul(puvw[:, 2 * H:2 * H + D], lhsT=hsl,
                                         rhs=we1_sb[:], start=True, stop=True)
                        nc.scalar.copy(u_sb[:, b * H:(b + 1) * H],
                                       puvw[:, 0:H])
                        vwst = smp.tile([P, 256], bf16, tag="vwst", bufs=2)
                        nc.vector.tensor_copy(vwst[:, 0:192],
                                              puvw[:, H:2 * H + D])
                        nc.gpsimd.memset(vwst[:, 192:256], 0.0)
                        nc.sync.dma_start(
                            vw_own[pb][b * P:(b + 1) * P, :], vwst[:])

                # ---- share the src-side table
                if VARIANT not in ("tablesonly", "nocoll"):
                    nc.gpsimd.collective_compute(
                        "AllGather", AL.bypass,
                        replica_groups=[list(range(cfg.ncores))],
                        ins=[vw_own[pb][:]], outs=[vw_full[pb][:]])

                # ---- edge pass
                vw_col = 0
                for w_i, w in enumerate(meta["win"] if VARIANT not in
                                        ("tablesonly", "tabag") else []):
                    wb = w["blocks"]
                    nwb = len(wb)
                    T_w = len(w["tiles"])
                    t_lo = w["s_lo"] // P
                    t_hi = w["s_hi"] // P
                    s_all = w["s_lo"] + w["s_hi"]
                    te0 = w["tile0"]

                    # gathers
                    vg = wnp.tile([P, max_tw, 256], bf16, tag="vg", bufs=4)
                    t_a = t_lo // 2
                    s_a = t_a * P
                    if t_a:
                        nc.gpsimd.dma_gather(
                            out_ap=vg[:, 0:t_a, :], in_ap=vw_full[pb][:],
                            idxs_ap=vwidx[:, vw_col:vw_col + s_a // 16],
                            num_idxs=s_a, num_idxs_reg=s_a,
                            elem_size=256, single_packet=False,
                            queue_num=(3 * w_i) % 4)
                    nc.gpsimd.dma_gather(
                        out_ap=vg[:, t_a:t_lo, :], in_ap=vw_full[pb][:],
                        idxs_ap=vwidx[:, vw_col + s_a // 16:
                                      vw_col + w["s_lo"] // 16],
                        num_idxs=w["s_lo"] - s_a,
                        num_idxs_reg=w["s_lo"] - s_a,
                        elem_size=256, single_packet=False,
                        queue_num=(3 * w_i + 1) % 4)
                    vw_col += w["s_lo"] // 16
                    if t_hi:
                        nc.gpsimd.dma_gather(
                            out_ap=vg[:, t_lo:T_w, :],
                            in_ap=vw_full[pb][cfg.half:, :],
                            idxs_ap=vwidx[:, vw_col:vw_col + w["s_hi"] // 16],
                            num_idxs=w["s_hi"], num_idxs_reg=w["s_hi"],
                            elem_size=256, single_packet=False,
                            queue_num=(3 * w_i + 2) % 4)
                        vw_col += w["s_hi"] // 16
                    # edge_attr columns into the gathered rows' pad so the
                    # scatter matmul consumes [w | ea] in one rhs stream.
                    # (reads vg's pad back through in1 to order after the
                    # gathers -- plain WAW on the custom gather is not enough)
                    nc.vector.scalar_tensor_tensor(
                        out=vg[:, 0:T_w, 192:194],
                        in0=ea[:, 2 * te0:2 * (te0 + T_w)]
                        .rearrange("p (t e) -> p t e", e=2),
                        scalar=0.0,
                        in1=vg[:, 0:T_w, 192:194],
                        op0=AL.add, op1=AL.bypass)
                    if VARIANT == "gonly":
                        continue

                    # S1T for the whole window: broadcast offs stripe, compare
                    p0row = wnp.tile([1, max_sall], bf16, tag="p0row", bufs=2)
                    nc.sync.dma_start(p0row[0:1, 0:s_all],
                                      offsf_d[0:1, w["slot0"]:w["slot0"] + s_all])
                    brow = wnp.tile([P, max_sall], bf16, tag="brow", bufs=2)
                    nc.gpsimd.partition_broadcast(brow[:, 0:s_all],
                                                  p0row[0:1, 0:s_all])
                    s1t = wnp.tile([P, max_sall], bf16, tag="s1t")
                    nc.vector.tensor_scalar(
                        out=s1t[:, 0:s_all], in0=brow[:, 0:s_all],
                        scalar1=iotac[:], scalar2=None, op0=AL.is_equal)

                    # u[dst] + attention, chunked 8 tiles per PSUM bank
                    logit = wnp.tile([P, max_tw], bf16, tag="logit")
                    for c0 in range(0, T_w, 8):
                        cs = min(8, T_w - c0)
                        ugps = psB.tile([P, 512], f32, space="PSUM", tag="ug")
                        if VARIANT != "nou":
                            for i in range(cs):
                                ti = c0 + i
                                b = w["tiles"][ti]
                                nc.tensor.matmul(
                                    ugps[:, i * H:(i + 1) * H],
                                    lhsT=s1t[:, ti * P:(ti + 1) * P],
                                    rhs=u_sb[:, b * H:(b + 1) * H],
                                    start=True, stop=True)
                        else:
                            nc.vector.memset(ugps[:, 0:cs * H], 0.0)
                        a1 = wnp.tile([P, 8, H], bf16, tag="a1")
                        nc.vector.tensor_tensor(
                            out=a1[:, 0:cs, :],
                            in0=ugps[:, 0:cs * H]
                            .rearrange("p (t h) -> p t h", h=H),
                            in1=vg[:, c0:c0 + cs, 0:H], op=AL.add)
                        rw = wnp.tile([P, 8, H], bf16, tag="rw", bufs=2)
                        w2b = w2r_sb[:].rearrange("p (t h) -> p t h", t=1) \
                                       .broadcast_to((P, cs, H))
                        nc.vector.scalar_tensor_tensor(
                            out=rw[:, 0:cs, :], in0=a1[:, 0:cs, :], scalar=0.0,
                            in1=w2b, op0=AL.max, op1=AL.mult)
                        with nc.allow_low_precision(
                                reason="bf16 logits feed sigmoid"):
                            nc.vector.tensor_reduce(
                                out=logit[:, c0:c0 + cs], in_=rw[:, 0:cs, :],
                                axis=mybir.AxisListType.X, op=AL.add)
                    att = wnp.tile([P, max_tw], f32, tag="att")
                    nc.scalar.activation(att[:, 0:T_w], logit[:, 0:T_w],
                                         AF.Sigmoid)

                    # scatter (edge_attr factor rides as 2 extra rhs columns)
                    accs = []
                    for _bi in range(nwb):
                        accb = psA.tile([P, 512], f32, space="PSUM",
                                        tag="acc")
                        accs.append(accb)
                    for ti, b in enumerate(w["tiles"]):
                        gt = te0 + ti
                        bl = wb.index(b)
                        first = gt == meta["blk_tiles"][b][0]
                        last = gt == meta["blk_tiles"][b][-1]
                        satt = sap.tile([P, P], bf16, tag="satt")
                        eng = nc.vector if ti % 2 == 0 else nc.gpsimd
                        eng.tensor_scalar(
                            out=satt[:], in0=iota[:],
                            scalar1=offs[:, gt:gt + 1],
                            scalar2=att[:, ti:ti + 1],
                            op0=AL.is_equal, op1=AL.mult)
                        nc.tensor.matmul(
                            accs[bl][:, 0:D + 2], lhsT=satt[:],
                            rhs=vg[:, ti, H:H + D + 2],
                            start=first, stop=last)

                    # per-block: eat = P2 @ We2 on DVE; osl; bn stats
                    for bl, b in enumerate(wb):
                        acc = accs[bl]
                        p2 = smp.tile([P, 2], f32, tag="p2")
                        nc.vector.tensor_copy(p2[:], acc[:, D:D + 2])
                        e0 = smp.tile([P, D], f32, tag="e0", bufs=2)
                        nc.scalar.activation(
                            e0[:], we2b_sb[:, 0:D], AF.Copy,
                            scale=p2[:, 0:1])
                        e1 = smp.tile([P, D], f32, tag="e1", bufs=2)
                        nc.scalar.activation(
                            e1[:], we2b_sb[:, D:2 * D], AF.Copy,
                            scale=p2[:, 1:2])
                        t3 = smp.tile([P, D], f32, tag="t3", bufs=2)
                        nc.vector.tensor_tensor(
                            out=t3[:], in0=acc[:, 0:D], in1=e0[:], op=AL.add)
                        osl = osl_all[:, b * D:(b + 1) * D]
                        nc.vector.tensor_tensor(
                            out=osl, in0=t3[:], in1=e1[:], op=AL.add)
                        st6 = smp.tile([P, 6], f32, tag="st6")
                        nc.vector.bn_stats(st6[:], osl)
                        nc.vector.bn_aggr(mv[:, 2 * b:2 * b + 2], st6[:])

                # ---- deferred LN apply + relu + residual
                if VARIANT in ("tablesonly", "tabag", "gonly"):
                    continue
                mv3 = mv[:].rearrange("p (b t) -> p b t", t=2)
                std49 = smp.tile([P, bpc], f32, tag="std49")
                nc.scalar.activation(std49[:], mv3[:, :, 1:2], AF.Sqrt,
                                     bias=eps_sb[:])
                rstd49 = smp.tile([P, bpc], f32, tag="rstd49")
                nc.vector.reciprocal(rstd49[:], std49[:])
                for b in range(bpc):
                    lnw = smp.tile([P, D], f32, tag="lnw")
                    nc.vector.tensor_scalar(
                        out=lnw[:], in0=osl_all[:, b * D:(b + 1) * D],
                        scalar1=mv[:, 2 * b:2 * b + 1],
                        scalar2=rstd49[:, b:b + 1],
                        op0=AL.subtract, op1=AL.mult)
                    x_sl = xrows[:, b * D:(b + 1) * D]
                    if l < L - 1:
                        nc.vector.scalar_tensor_tensor(
                            out=x_sl, in0=lnw[:], scalar=0.0, in1=x_sl,
                            op0=AL.max, op1=AL.add)
                    else:
                        nc.vector.tensor_tensor(out=x_sl, in0=lnw[:],
                                                in1=x_sl, op=AL.add)

            nc.sync.dma_start(y_d[:], xrows[:])

    nc.compile()
    return nc


# ---------------------------------------------------------------- entry point
def make_in_maps(cfg, meta, percore, core_in, inputs):
    edge_attr = np.asarray(inputs["edge_attr"], dtype=np.float32)
    lin_node_w = np.asarray(inputs["lin_node_w"], dtype=np.float32)
    lin_edge_w = np.asarray(inputs["lin_edge_w"], dtype=np.float32)
    att_w1 = np.asarray(inputs["att_w1"], dtype=np.float32)
    att_w2 = np.asarray(inputs["att_w2"], dtype=np.float32)
    L = cfg.L
    wn = lin_node_w.astype(BF)
    w1a = att_w1[:, :D, :].astype(BF)
    w1b = att_w1[:, D:, :].astype(BF)
    we1 = lin_edge_w[:, :D, :].astype(BF)
    we2 = lin_edge_w[:, D:, :]                       # [L, 2, D]
    we2b = np.broadcast_to(we2[:, None, :, :], (L, P, 2, D)) \
        .reshape(L, P, 2 * D).astype(BF)
    w2r = np.broadcast_to(att_w2[:, :, 0][:, None, :], (L, P, H)).astype(BF)
    ident = np.eye(P, dtype=np.float32)
    iota = np.broadcast_to(np.arange(P, dtype=np.float32), (P, P)).astype(BF)
    iotac = np.arange(P, dtype=np.float32).reshape(P, 1)
    tt = meta["tot_tiles"]
    in_maps = []
    for c in range(cfg.ncores):
        ea_slots = np.zeros((tt * P, 2), dtype=np.float32)
        valid = percore[c]["eslot"] >= 0
        ea_slots[valid] = edge_attr[percore[c]["eslot"][valid]]
        ea2 = ea_slots.reshape(tt, P, 2).transpose(1, 0, 2).reshape(P, 2 * tt)
        in_maps.append(dict(
            xrows=core_in[c]["xrows"],
            vwidx=core_in[c]["vwidx"].astype(np.int16),
            offs=core_in[c]["offs"].astype(np.float32),
            offs_flat=core_in[c]["offs_flat"],
            ea2=np.ascontiguousarray(ea2).astype(BF),
            wn=wn, w1a=w1a, w1b=w1b, we1=we1, we2b=we2b, w2r=w2r,
            ident=ident, iota=iota, iotac=iotac,
        ))
    return in_maps


def run(cfg, inputs, nc=None):
    edge_index = np.asarray(inputs["edge_index"])
    x = np.asarray(inputs["x"], dtype=np.float32)

    for name in ("att_b1", "att_b2", "bias", "ln_beta"):
        assert not np.any(np.asarray(inputs[name])), f"{name} must be zero"
    assert np.all(np.asarray(inputs["ln_gamma"]) == 1.0), "ln_gamma must be 1"

    percore, meta = prep_edges(cfg, edge_index)
    x_pad = np.zeros((cfg.npad, D), dtype=np.float32)
    x_pad[:cfg.N] = x
    core_in = build_core_inputs(cfg, meta, percore, x_pad)
    in_maps = make_in_maps(cfg, meta, percore, core_in, inputs)

    if nc is None:
        nc = build_nc(cfg, meta)
    res = bass_utils.run_bass_kernel_spmd(
        nc, in_maps, core_ids=list(range(cfg.ncores)))
    outs = []
    for c in range(cfg.ncores):
        yr = res.results[c]["y"]                     # [128, bpc*128]
        outs.append(yr.reshape(P, cfg.bpc, D).transpose(1, 0, 2)
                    .reshape(cfg.nodes_pc, D))
    full = np.concatenate(outs, axis=0)[:cfg.N]
    return np.ascontiguousarray(full), nc, in_maps, meta


def kernel(**inputs) -> np.ndarray:
    out, _, _, _ = run(REAL, inputs)
    return out
